# revision 13
# baseline (speedup 1.0000x reference)
"""Trainium2 Bass kernel for nn_CorrBlockSingleScale (RAFT single-scale
correlation lookup), distributed over 8 NeuronCores.

  fmap1, fmap2: [1, 256, 64, 96] f32;  coords: [1, 2, 64, 96] f32; radius=4
  corr = einsum('bcm,bcn->bmn', f1, f2) / 16        -> [6144, 64, 96]
  out[q, i, j] = bilinear(corr[q], (cx_q + d_i, cy_q + d_j)),  d in -4..4
  output [1, 81, 64, 96] f32.

Structure exploited: the 9x9 sample offsets are integers, so all 81 samples
of a query share one fractional pair (fx, fy) -- the output is a separable
2x2-tap blend of a 10x10 patch of corr[q] anchored at
(floor(cx)-4, floor(cy)-4).

Distribution / tiling (no collectives): queries sorted by floor(cy),
chopped into 8 cores of 768; within a core re-sorted by floor(cx) and
chopped into 6 tiles of 128, so each tile's band is only BX_t x SROWS
cells (~32 x 18) of the target plane.  Per-tile x-anchors are unified
across cores (max extent) so one SPMD program serves all 8 cores.

Per core:
  1. bf16 matmul f1_tile^T @ f2_band (K=256 = 2 accumulating matmuls)
     producing the x-major band [128, BX*SROWS] in PSUM.  The f2 slab is
     zero-padded (x in [-5,101), y rows [ys, ys+SROWS)) so out-of-plane
     bilinear taps read stored zeros -- no masks.
  2. cast band to bf16, DMA to DRAM scratch, indirect-DMA gather 10 full
     x-strips per query at *strip* granularity (descriptor-friendly: the
     window start index is a strip number, not an element).
  3. resolve the per-query y offset (0..SROWS-10) with a log-step barrel
     shift: tensor_copy (ACT/Pool) + copy_predicated (DVE) per bit, with
     per-partition mask scalars broadcast along the free axis.
  4. separable bilinear blend with host-folded weights (2 ACT muls +
     2 DVE scalar_tensor_tensor FMAs); bf16 output rows, one DMA at end.
Host post-pass upcasts and inverse-permutes rows to reference layout.
"""

import math

import numpy as np
import ml_dtypes

import concourse.bass as bass
import concourse.bacc as bacc
import concourse.mybir as mybir
import concourse.tile as tile
from concourse import bass_utils
from concourse.bass import broadcast_tensor_aps

BF16NP = ml_dtypes.bfloat16
F32 = mybir.dt.float32
I32 = mybir.dt.int32
BF16 = mybir.dt.bfloat16

B, C, H, W = 1, 256, 64, 96
R = 4
K = 2 * R + 1          # 9
PK = K + 1             # 10 (patch side)
NQ = H * W             # 6144
NCORES = 8
QPC = NQ // NCORES     # 768
P = 128
NT = QPC // P          # 6 tiles per core
KH = 2                 # K halves (256 = 2 x 128)
PADX = 5               # padded x coords [-5, 101)
PADY = 5               # padded y coords [-5, 69)
WPAD = W + 2 * PADX    # 106
def _nstages(srows):
    dymax = srows - PK
    n = 0
    while (1 << n) - 1 < dymax:
        n += 1
    return max(n, 1)


# --------------------------------------------------------------------------
# host-side preprocessing
# --------------------------------------------------------------------------

def host_preprocess(fmap1, fmap2, coords):
    """Returns (in_maps, order, geom) with geom = (SROWS, BX tuple, ax tuple)."""
    f1 = np.asarray(fmap1, np.float32).reshape(C, NQ)
    f2 = np.asarray(fmap2, np.float32).reshape(C, H, W)
    cx_all = np.asarray(coords, np.float32)[0, 0].reshape(NQ)
    cy_all = np.asarray(coords, np.float32)[0, 1].reshape(NQ)
    ix_all = np.floor(cx_all).astype(np.int64)
    iy_all = np.floor(cy_all).astype(np.int64)

    yorder = np.lexsort((np.arange(NQ), ix_all, iy_all))
    order = np.empty(NQ, np.int64)
    for c in range(NCORES):
        qs = yorder[c * QPC:(c + 1) * QPC]
        sub = np.lexsort((np.arange(QPC), iy_all[qs], ix_all[qs]))
        order[c * QPC:(c + 1) * QPC] = qs[sub]

    plane = np.zeros((C, H + 2 * PADY, WPAD), np.float32)
    plane[:, PADY:PADY + H, PADX:PADX + W] = f2
    plane = plane.astype(BF16NP)

    cores = []
    for c in range(NCORES):
        qs = order[c * QPC:(c + 1) * QPC]
        jx = ix_all[qs]
        jy = iy_all[qs]
        fx = (cx_all[qs] - jx).astype(np.float32)
        fy = (cy_all[qs] - jy).astype(np.float32)
        cores.append(dict(qs=qs, jx=jx, jy=jy, fx=fx, fy=fy))

    SROWS = max(int(c["jy"].max() - c["jy"].min()) + PK for c in cores)
    for c in cores:
        ys = int(c["jy"].min()) - R
        ys = max(min(ys, H + PADY - SROWS), -PADY)
        c["ys"] = ys
        assert ys <= c["jy"].min() - R
        assert ys + SROWS >= c["jy"].max() + R + 2

    ax, BX = [], []
    for t in range(NT):
        sel = slice(t * P, (t + 1) * P)
        lo = min(int(c["jx"][sel].min()) for c in cores) - R
        hi = max(int(c["jx"][sel].max()) for c in cores) + R + 1
        ax.append(lo)
        BX.append(hi - lo + 1)

    nst = _nstages(SROWS)
    f1b = f1.astype(BF16NP)
    in_maps = []
    for c in cores:
        qs = c["qs"]
        jx, jy, ys = c["jx"], c["jy"], c["ys"]
        f1s = np.ascontiguousarray(f1b[:, qs].reshape(KH, P, QPC))
        slab = np.ascontiguousarray(
            plane[:, ys + PADY: ys + PADY + SROWS, :].reshape(KH, P, SROWS * WPAD))

        meta = np.zeros((QPC, 5 + nst), np.int32)
        for t in range(NT):
            sel = slice(t * P, (t + 1) * P)
            dx = jx[sel] - R - ax[t]
            dy = jy[sel] - R - ys
            assert (dx >= 0).all() and (dx + PK <= BX[t]).all()
            assert (dy >= 0).all() and (dy + PK <= SROWS).all()
            meta[sel, 0] = (np.arange(P) * BX[t] + dx).astype(np.int32)
            wy0 = (1.0 - c["fy"][sel]).astype(np.float32)
            wy1 = c["fy"][sel].astype(np.float32)
            wx0 = ((1.0 - c["fx"][sel]) / 16.0).astype(np.float32)
            wx1 = (c["fx"][sel] / 16.0).astype(np.float32)
            meta[sel, 1] = wy0.view(np.int32)
            meta[sel, 2] = wy1.view(np.int32)
            meta[sel, 3] = wx0.view(np.int32)
            meta[sel, 4] = wx1.view(np.int32)
            for s in range(nst):
                bit = nst - 1 - s               # stage s shifts by 2^bit
                meta[sel, 5 + s] = ((dy >> bit) & 1).astype(np.int32)

        in_maps.append({"f1s": f1s, "f2s": slab, "meta": meta})

    geom = (SROWS, tuple(BX), tuple(ax))
    return in_maps, order, geom


def assemble_output(results, order):
    rows = np.concatenate(
        [results[c]["out"].astype(np.float32) for c in range(NCORES)], axis=0)
    full = np.empty((K * K, NQ), np.float32)
    full[:, order] = rows.T
    return full.reshape(1, K * K, H, W)


# --------------------------------------------------------------------------
# device program
# --------------------------------------------------------------------------

def _body(tc, nc, aps, scr, geom):
    SROWS, BX, ax = geom
    nst = _nstages(SROWS)
    NM = 5 + nst
    GW = PK * SROWS            # gathered window (10 full x-strips)
    FW = (PK - 1) * SROWS + PK  # valid prefix after barrel
    import contextlib
    ctx = contextlib.ExitStack()
    with ctx:
        const = ctx.enter_context(tc.tile_pool(name="const", bufs=1))
        corr_pool = ctx.enter_context(tc.tile_pool(name="corr", bufs=3))
        psum_pool = ctx.enter_context(
            tc.tile_pool(name="ps", bufs=4, space="PSUM"))
        small = ctx.enter_context(tc.tile_pool(name="small", bufs=3))

        f1sb = const.tile([P, KH * QPC], BF16)
        nc.sync.dma_start(
            f1sb[:].rearrange("p (k m) -> p k m", k=KH),
            aps["f1s"].rearrange("k p m -> p k m"))
        f2sb = const.tile([P, KH * SROWS * WPAD], BF16)
        nc.sync.dma_start(
            f2sb[:].rearrange("p (k m) -> p k m", k=KH),
            aps["f2s"].rearrange("k p m -> p k m"))
        metab = const.tile([P, NT * NM], I32)
        nc.sync.dma_start(
            metab[:].rearrange("p (t a) -> p t a", a=NM),
            aps["meta"].rearrange("(t p) a -> p t a", p=P))
        otall = const.tile([P, NT * K * K], BF16)

        # [p, kh, x(col, stride 1), r(row, stride WPAD)]
        f2v = f2sb[:].rearrange("p (k r x) -> p k x r", k=KH, x=WPAD)

        cbx_max = max(1, 512 // SROWS)
        for t in range(NT):
            bxt = BX[t]
            corr_sb = corr_pool.tile([P, bxt * SROWS], BF16, tag="corr")
            x0 = 0
            ci = 0
            while x0 < bxt:
                cbx = min(cbx_max, bxt - x0)
                cw = cbx * SROWS
                ps = psum_pool.tile([P, 512], F32, space="PSUM", tag="ps")
                for kh in range(KH):
                    lhsT = f1sb[:, kh * QPC + t * P: kh * QPC + (t + 1) * P]
                    rhs = f2v[:, kh, ax[t] + PADX + x0:
                              ax[t] + PADX + x0 + cbx, :]
                    nc.tensor.matmul(ps[:, :cw], lhsT=lhsT, rhs=rhs,
                                     start=(kh == 0), stop=(kh == KH - 1))
                dst = corr_sb[:, x0 * SROWS: x0 * SROWS + cw]
                if ci % 2 == 0:
                    nc.scalar.copy(dst, ps[:, :cw])
                else:
                    nc.vector.tensor_copy(dst, ps[:, :cw])
                x0 += cbx
                ci += 1

            sdst = scr[t].ap()[0: P * bxt * SROWS].rearrange(
                "(p f) -> p f", p=P)
            nc.sync.dma_start(sdst, corr_sb[:])

            # gather 10 x-strips per query at strip granularity
            pa = small.tile([P, GW], BF16, tag="pa")
            src = scr[t].ap().rearrange("(n r) -> n r", r=SROWS)
            nc.gpsimd.indirect_dma_start(
                out=pa[:, 0:GW], out_offset=None, in_=src,
                in_offset=bass.IndirectOffsetOnAxis(
                    ap=metab[:, NM * t: NM * t + 1], axis=0))

            # y-offset barrel shift (in-SBUF, per-partition masks)
            pb = small.tile([P, GW], BF16, tag="pb")
            bufs = [pa, pb]
            cur = 0
            for s in range(nst):
                v = 1 << (nst - 1 - s)
                src_t, dst_t = bufs[cur], bufs[1 - cur]
                if s % 2 == 0:
                    nc.gpsimd.tensor_copy(dst_t[:], src_t[:])
                else:
                    nc.scalar.copy(dst_t[:], src_t[:])
                mcol = metab[:, NM * t + 5 + s: NM * t + 6 + s]
                shifted = src_t[:, v:GW]
                mask_b, _ = broadcast_tensor_aps(mcol, shifted)
                nc.vector.copy_predicated(dst_t[:, 0:GW - v], mask_b, shifted)
                cur = 1 - cur
            pt = bufs[cur]
            ptv = pt[:].rearrange("p (b r) -> p b r", r=SROWS)[:, :, 0:PK]

            def wap(a):
                return metab[:, NM * t + 1 + a: NM * t + 2 + a].bitcast(F32)

            t1 = small.tile([P, PK * K], F32, tag="t1")
            t13 = t1[:].rearrange("p (a b) -> p a b", b=K)
            nc.scalar.mul(t13, ptv[:, :, 1:PK], wap(1))
            cm = small.tile([P, PK * K], F32, tag="cm")
            cm3 = cm[:].rearrange("p (a b) -> p a b", b=K)
            nc.vector.scalar_tensor_tensor(
                cm3, ptv[:, :, 0:K], wap(0), t13,
                op0=mybir.AluOpType.mult, op1=mybir.AluOpType.add)

            t2 = small.tile([P, K * K], F32, tag="t2")
            t23 = t2[:].rearrange("p (a b) -> p a b", b=K)
            nc.scalar.mul(t23, cm3[:, 1:PK, :], wap(3))
            ot3 = otall[:, t * K * K: (t + 1) * K * K].rearrange(
                "p (a b) -> p a b", b=K)
            nc.vector.scalar_tensor_tensor(
                ot3, cm3[:, 0:K, :], wap(2), t23,
                op0=mybir.AluOpType.mult, op1=mybir.AluOpType.add)

        nc.sync.dma_start(
            aps["out"].rearrange("(t p) k -> p t k", p=P),
            otall[:].rearrange("p (t k) -> p t k", k=K * K))


def build_program(geom, rep=1):
    """rep>1 wraps the body in a For_i loop (for wall-clock timing)."""
    SROWS, BX, ax = geom
    nst = _nstages(SROWS)
    nc = bacc.Bacc("TRN2", target_bir_lowering=False, debug=False,
                   num_devices=NCORES)
    aps = {}
    aps["f1s"] = nc.dram_tensor("f1s", [KH, P, QPC], BF16,
                                kind="ExternalInput").ap()
    aps["f2s"] = nc.dram_tensor("f2s", [KH, P, SROWS * WPAD], BF16,
                                kind="ExternalInput").ap()
    aps["meta"] = nc.dram_tensor("meta", [QPC, 5 + nst], I32,
                                 kind="ExternalInput").ap()
    aps["out"] = nc.dram_tensor("out", [QPC, K * K], BF16,
                                kind="ExternalOutput").ap()
    scr = [nc.dram_tensor(f"scr{t}", [P * BX[t] * SROWS], BF16)
           for t in range(NT)]

    with tile.TileContext(nc) as tc:
        if rep == 1:
            _body(tc, nc, aps, scr, geom)
        else:
            with tc.For_i(0, rep):
                _body(tc, nc, aps, scr, geom)
    nc.compile()
    return nc


_PROGRAMS = {}


def kernel(fmap1, fmap2, coords, radius):
    assert int(radius) == R, f"kernel hardcodes radius=4, got {radius}"
    in_maps, order, geom = host_preprocess(fmap1, fmap2, coords)
    nc = _PROGRAMS.get(geom)
    if nc is None:
        nc = _PROGRAMS[geom] = build_program(geom)
    last_err = None
    for _ in range(3):  # the remote compile hook occasionally flakes
        try:
            res = bass_utils.run_bass_kernel_spmd(
                nc, in_maps, core_ids=list(range(NCORES)))
            return assemble_output(res.results, order)
        except Exception as e:  # noqa: BLE001
            last_err = e
    raise last_err


# revision 14
# speedup vs baseline: 1.9721x; 1.9721x over previous
"""Trainium2 Bass kernel for nn_CorrBlockSingleScale (RAFT single-scale
correlation lookup), distributed over 8 NeuronCores.

  fmap1, fmap2: [1, 256, 64, 96] f32;  coords: [1, 2, 64, 96] f32; radius=4
  corr = einsum('bcm,bcn->bmn', f1, f2) / 16        -> [6144, 64, 96]
  out[q, i, j] = bilinear(corr[q], (cx_q + d_i, cy_q + d_j)),  d in -4..4
  output [1, 81, 64, 96] f32.

Structure exploited: the 9x9 sample offsets are integers, so all 81 samples
of a query share one fractional pair (fx, fy) -- the output is a separable
2x2-tap blend of a 10x10 patch of corr[q] anchored at
(floor(cx)-4, floor(cy)-4).

Distribution / tiling (no collectives): queries sorted by floor(cy),
chopped into 8 cores of 768; within a core re-sorted by floor(cx) and
chopped into 6 tiles of 128, so each tile's correlation band is only
BXF x SROWS cells (~30 x 18) of the 64x96 target plane.  The host ships
per-(core,tile) pre-sliced bf16 bands cut from a zero-padded plane
(x in [-5,101), y rows [ys, ys+SROWS)), so out-of-plane bilinear taps
read stored zeros -- no masks, and no cross-core anchor unification.

Per core and iteration (35 device instructions total):
  1. per tile: one accumulating bf16 matmul pair (K=256 = 2x128) into one
     PSUM bank (band <= 512 elems), cast to bf16 into a slice of one big
     corr tile (ACT/DVE alternating).
  2. ONE scratch write DMA [128, 6*BXF*SROWS] -> DRAM.
  3. per tile: indirect-DMA gather of each query's contiguous 172-element
     window (the 10x10 patch in the x-major band) into a slice of one
     patch tile.
  4. separable bilinear blend for ALL tiles at once: 6 tensor_tensor ops
     with per-(partition,tile) weights broadcast along the patch axes
     (stride-0 APs).  bf16 output rows, one DMA.
Host post-pass upcasts and inverse-permutes rows to reference layout.
"""

import contextlib

import numpy as np
import ml_dtypes

import concourse.bass as bass
import concourse.bacc as bacc
import concourse.mybir as mybir
import concourse.tile as tile
from concourse import bass_utils
from concourse.bass import broadcast_tensor_aps

BF16NP = ml_dtypes.bfloat16
F32 = mybir.dt.float32
I32 = mybir.dt.int32
BF16 = mybir.dt.bfloat16

B, C, H, W = 1, 256, 64, 96
R = 4
K = 2 * R + 1          # 9
PK = K + 1             # 10 (patch side)
NQ = H * W             # 6144
NCORES = 8
QPC = NQ // NCORES     # 768
P = 128
NT = QPC // P          # 6 tiles per core
KH = 2                 # K halves (256 = 2 x 128)
PADX = 5               # padded x coords [-5, 101)
PADY = 5               # padded y coords [-5, 69)
WPAD = W + 2 * PADX    # 106
NM = 5                 # meta cols: idx, wy0, wy1, wx0/16, wx1/16


# --------------------------------------------------------------------------
# host-side preprocessing
# --------------------------------------------------------------------------

def host_preprocess(fmap1, fmap2, coords):
    """Returns (in_maps, order, geom) with geom = (SROWS, BXF)."""
    f1 = np.asarray(fmap1, np.float32).reshape(C, NQ)
    f2 = np.asarray(fmap2, np.float32).reshape(C, H, W)
    cx_all = np.asarray(coords, np.float32)[0, 0].reshape(NQ)
    cy_all = np.asarray(coords, np.float32)[0, 1].reshape(NQ)
    ix_all = np.floor(cx_all).astype(np.int64)
    iy_all = np.floor(cy_all).astype(np.int64)

    yorder = np.lexsort((np.arange(NQ), ix_all, iy_all))
    order = np.empty(NQ, np.int64)
    for c in range(NCORES):
        qs = yorder[c * QPC:(c + 1) * QPC]
        sub = np.lexsort((np.arange(QPC), iy_all[qs], ix_all[qs]))
        order[c * QPC:(c + 1) * QPC] = qs[sub]

    plane = np.zeros((C, H + 2 * PADY, WPAD), np.float32)
    plane[:, PADY:PADY + H, PADX:PADX + W] = f2
    plane = plane.astype(BF16NP)

    cores = []
    for c in range(NCORES):
        qs = order[c * QPC:(c + 1) * QPC]
        jx = ix_all[qs]
        jy = iy_all[qs]
        fx = (cx_all[qs] - jx).astype(np.float32)
        fy = (cy_all[qs] - jy).astype(np.float32)
        cores.append(dict(qs=qs, jx=jx, jy=jy, fx=fx, fy=fy))

    SROWS = max(int(c["jy"].max() - c["jy"].min()) + PK for c in cores)
    for c in cores:
        ys = int(c["jy"].min()) - R
        ys = max(min(ys, H + PADY - SROWS), -PADY)
        c["ys"] = ys
        assert ys <= c["jy"].min() - R
        assert ys + SROWS >= c["jy"].max() + R + 2

    # per-(core,tile) x anchors; uniform band width = max extent
    BXF = 0
    for c in cores:
        axs = []
        for t in range(NT):
            sel = slice(t * P, (t + 1) * P)
            lo = int(c["jx"][sel].min()) - R
            hi = int(c["jx"][sel].max()) + R + 1
            axs.append((lo, hi - lo + 1))
            BXF = max(BXF, hi - lo + 1)
        c["axs"] = axs
    for c in cores:
        c["ax"] = [max(min(lo, W + PADX - BXF), -PADX)
                   for lo, _ in c["axs"]]
        for t in range(NT):
            sel = slice(t * P, (t + 1) * P)
            assert c["ax"][t] <= c["jx"][sel].min() - R
            assert c["ax"][t] + BXF >= c["jx"][sel].max() + R + 2

    BXFS = BXF * SROWS
    f1b = f1.astype(BF16NP)
    in_maps = []
    for c in cores:
        qs, jx, jy, ys = c["qs"], c["jx"], c["jy"], c["ys"]
        f1s = np.ascontiguousarray(f1b[:, qs].reshape(KH, P, QPC))

        bands = np.empty((NT, KH, P, BXFS), BF16NP)
        for t in range(NT):
            colsel = plane[:, ys + PADY: ys + PADY + SROWS,
                           c["ax"][t] + PADX: c["ax"][t] + PADX + BXF]
            bands[t] = colsel.transpose(0, 2, 1).reshape(KH, P, BXFS)

        meta = np.zeros((QPC, NM), np.int32)
        for t in range(NT):
            sel = slice(t * P, (t + 1) * P)
            dx = jx[sel] - R - c["ax"][t]
            dy = jy[sel] - R - ys
            idx = (np.arange(P) * (NT * BXFS) + t * BXFS
                   + dx * SROWS + dy)
            meta[sel, 0] = idx.astype(np.int32)
            meta[sel, 1] = (1.0 - c["fy"][sel]).astype(np.float32) \
                .view(np.int32)
            meta[sel, 2] = c["fy"][sel].astype(np.float32).view(np.int32)
            meta[sel, 3] = ((1.0 - c["fx"][sel]) / 16.0) \
                .astype(np.float32).view(np.int32)
            meta[sel, 4] = (c["fx"][sel] / 16.0).astype(np.float32) \
                .view(np.int32)

        in_maps.append({"f1s": f1s, "f2s": np.ascontiguousarray(bands),
                        "meta": meta})

    geom = (SROWS, BXF)
    return in_maps, order, geom


def assemble_output(results, order):
    rows = np.concatenate(
        [results[c]["out"].astype(np.float32) for c in range(NCORES)], axis=0)
    full = np.empty((K * K, NQ), np.float32)
    full[:, order] = rows.T
    return full.reshape(1, K * K, H, W)


# --------------------------------------------------------------------------
# device program
# --------------------------------------------------------------------------

def _body(tc, nc, aps, scr, geom, pools, u):
    SROWS, BXF = geom
    BXFS = BXF * SROWS
    GW = PK * SROWS             # per-tile patch segment (gather fills 172)
    win = (PK - 1) * SROWS + PK
    const, corr_pool, psum_pool, small = pools

    f1sb = const.tile([P, KH * QPC], BF16, tag="f1sb")
    nc.sync.dma_start(
        f1sb[:].rearrange("p (k m) -> p k m", k=KH),
        aps["f1s"].rearrange("k p m -> p k m"))
    f2sb = const.tile([P, NT * KH * BXFS], BF16, tag="f2sb")
    nc.sync.dma_start(
        f2sb[:].rearrange("p (t k m) -> p t k m", t=NT, k=KH),
        aps["f2s"].rearrange("t k p m -> p t k m"))
    metab = const.tile([P, NT * NM], I32, tag="metab")
    nc.sync.dma_start(
        metab[:].rearrange("p (t a) -> p t a", a=NM),
        aps["meta"].rearrange("(t p) a -> p t a", p=P))
    m3 = metab[:].rearrange("p (t a) -> p t a", a=NM)

    corr_all = corr_pool.tile([P, NT * BXFS], BF16, tag="corr")
    nchunk = -(-BXFS // 512)
    cw = -(-BXF // nchunk) * SROWS
    for t in range(NT):
        x0 = 0
        ci = 0
        while x0 < BXFS:
            w = min(cw, BXFS - x0)
            ps = psum_pool.tile([P, 512], F32, space="PSUM", tag="ps")
            for kh in range(KH):
                lhsT = f1sb[:, kh * QPC + t * P: kh * QPC + (t + 1) * P]
                rhs = f2sb[:, (t * KH + kh) * BXFS + x0:
                           (t * KH + kh) * BXFS + x0 + w]
                nc.tensor.matmul(ps[:, :w], lhsT=lhsT, rhs=rhs,
                                 start=(kh == 0), stop=(kh == KH - 1))
            dst = corr_all[:, t * BXFS + x0: t * BXFS + x0 + w]
            if (t + ci) % 2 == 0:
                nc.scalar.copy(dst, ps[:, :w])
            else:
                nc.vector.tensor_copy(dst, ps[:, :w])
            x0 += w
            ci += 1

    sdst = scr.ap()[0: P * NT * BXFS].rearrange("(p f) -> p f", p=P)
    nc.sync.dma_start(sdst, corr_all[:])

    ptall = small.tile([P, NT * GW], BF16, tag="ptall")
    src = scr.ap().rearrange("(n o) -> n o", o=1)
    for t in range(NT):
        nc.gpsimd.indirect_dma_start(
            out=ptall[:, t * GW: t * GW + win], out_offset=None, in_=src,
            in_offset=bass.IndirectOffsetOnAxis(
                ap=metab[:, NM * t: NM * t + 1], axis=0))

    # broadcast per-(partition, tile) weights over the patch axes
    pt4 = ptall[:].rearrange("p (t b r) -> p t b r", b=PK, r=SROWS)

    def wb(a, like):
        wv = m3[:, :, 1 + a: 2 + a].bitcast(F32) \
            .rearrange("p t (x y) -> p t x y", x=1)
        return broadcast_tensor_aps(wv, like)[0]

    t1 = small.tile([P, NT * PK * K], F32, tag="t1")
    t14 = t1[:].rearrange("p (t a b) -> p t a b", t=NT, b=K)
    in_hi = pt4[:, :, :, 1:PK]
    nc.vector.tensor_tensor(t14, in_hi, wb(1, in_hi),
                            op=mybir.AluOpType.mult)
    u1 = small.tile([P, NT * PK * K], F32, tag="u1")
    u14 = u1[:].rearrange("p (t a b) -> p t a b", t=NT, b=K)
    in_lo = pt4[:, :, :, 0:K]
    nc.gpsimd.tensor_tensor(u14, in_lo, wb(0, in_lo),
                            op=mybir.AluOpType.mult)
    cm = small.tile([P, NT * PK * K], F32, tag="cm")
    nc.vector.tensor_tensor(cm[:], t1[:], u1[:], op=mybir.AluOpType.add)
    cm4 = cm[:].rearrange("p (t a b) -> p t a b", t=NT, b=K)

    t2 = small.tile([P, NT * K * K], F32, tag="t2")
    t24 = t2[:].rearrange("p (t a b) -> p t a b", t=NT, b=K)
    cm_hi = cm4[:, :, 1:PK, :]
    nc.vector.tensor_tensor(t24, cm_hi, wb(3, cm_hi),
                            op=mybir.AluOpType.mult)
    u2 = small.tile([P, NT * K * K], F32, tag="u2")
    u24 = u2[:].rearrange("p (t a b) -> p t a b", t=NT, b=K)
    cm_lo = cm4[:, :, 0:K, :]
    nc.gpsimd.tensor_tensor(u24, cm_lo, wb(2, cm_lo),
                            op=mybir.AluOpType.mult)
    otall = small.tile([P, NT * K * K], BF16, tag="otall")
    nc.vector.tensor_tensor(otall[:], t2[:], u2[:], op=mybir.AluOpType.add)

    nc.sync.dma_start(
        aps["out"].rearrange("(t p) k -> p t k", p=P),
        otall[:].rearrange("p (t k) -> p t k", k=K * K))


def build_program(geom, rep=1, unroll=1):
    """rep = number of For_i iterations; each runs `unroll` bodies."""
    SROWS, BXF = geom
    nc = bacc.Bacc("TRN2", target_bir_lowering=False, debug=False,
                   num_devices=NCORES)
    aps = {}
    aps["f1s"] = nc.dram_tensor("f1s", [KH, P, QPC], BF16,
                                kind="ExternalInput").ap()
    aps["f2s"] = nc.dram_tensor("f2s", [NT, KH, P, BXF * SROWS], BF16,
                                kind="ExternalInput").ap()
    aps["meta"] = nc.dram_tensor("meta", [QPC, NM], I32,
                                 kind="ExternalInput").ap()
    aps["out"] = nc.dram_tensor("out", [QPC, K * K], BF16,
                                kind="ExternalOutput").ap()
    scr = [nc.dram_tensor(f"scr{i}", [P * NT * BXF * SROWS], BF16)
           for i in range(min(2, unroll))]

    with tile.TileContext(nc) as tc:
        ctx = contextlib.ExitStack()
        with ctx:
            const = ctx.enter_context(tc.tile_pool(name="const", bufs=2))
            corr_pool = ctx.enter_context(tc.tile_pool(name="corr", bufs=2))
            psum_pool = ctx.enter_context(
                tc.tile_pool(name="ps", bufs=4, space="PSUM"))
            small = ctx.enter_context(tc.tile_pool(name="small", bufs=2))
            pools = (const, corr_pool, psum_pool, small)
            if rep == 1:
                for u in range(unroll):
                    _body(tc, nc, aps, scr[u % len(scr)], geom, pools, u)
            else:
                with tc.For_i(0, rep):
                    for u in range(unroll):
                        _body(tc, nc, aps, scr[u % len(scr)], geom, pools, u)
    nc.compile()
    return nc


_PROGRAMS = {}


def kernel(fmap1, fmap2, coords, radius):
    assert int(radius) == R, f"kernel hardcodes radius=4, got {radius}"
    in_maps, order, geom = host_preprocess(fmap1, fmap2, coords)
    nc = _PROGRAMS.get(geom)
    if nc is None:
        nc = _PROGRAMS[geom] = build_program(geom)
    last_err = None
    for _ in range(3):  # the remote compile hook occasionally flakes
        try:
            res = bass_utils.run_bass_kernel_spmd(
                nc, in_maps, core_ids=list(range(NCORES)))
            return assemble_output(res.results, order)
        except Exception as e:  # noqa: BLE001
            last_err = e
    raise last_err
